# revision 15
# baseline (speedup 1.0000x reference)
"""BitLinear (int4-fakequant x @ ternary-weight linear) Trainium2 Bass kernel.

Strassen variant. Math (per reference):
  maxabs[s] = max(|x[s, :]|) clamped to >= 1e-6
  q[s, k]   = round(x[s, k] / maxabs[s] * 7)           # in [-7, 7]
  xq        = q * maxabs / 7
  thresh    = 0.05 * mean(|w|)                          # global scalar
  sign[o,k] = 0 if |w[o,k]| < thresh else sign(w[o,k])  # in {-1, 0, 1}
  alpha[o]  = mean(|w[o, :]|)
  out[s, o] = (maxabs[s]/7) * alpha[o] * S[s,o] + bias[o],  S = q @ sign.T

S = A @ B with A = q [M, K] (ints in [-7,7]) and B = sign.T [K, O_SH] (ternary)
is computed with ONE level of Strassen: A, B split 2x2 into [M/2, K/2] and
[K/2, N/2] blocks, 7 products Mi instead of 8 -> 7/8 the PE-array work, which
is the binding resource (fp8 DoubleRow streams 1 col-pair/cycle = 157 TF/s;
the dense kernel measured 905 us vs the 874 us stream floor). Host precomputes
the O(n^2) part: int4/ternary quantization AND the Strassen input combos
(A11+A22 etc., |.|<=14; B combos |.|<=2 - all exactly representable in e4m3),
shipped pre-tiled to SBUF layout. |Mi| <= 28*2048 << 2^24 so fp32 PSUM
accumulation is EXACT; the C recombination is spread over Scalar (3 PSUM->SBUF
copies), GpSimd (1 SBUF add) and Vector (7 PSUM adds, ordered so each PSUM
bank frees just before the next half re-needs it), all overlapped with the PE
stream. Column-parallel over out_f across 8 cores.

Device per-core schedule (M=8192, K=4096, O_SH=2048):
  B combos (7 x 2 o-halves x [128, 8, 2, 512] e4m3 = 112 KB/partition) are
  SBUF-resident, loaded in first-use order. Loop over 32 row-blocks (128 top
  rows r*128.. paired with 128 bottom rows 4096+r*128..; A combos for both
  land as one 1.75 MB pre-tiled DMA). Per row-block, 2 o-halves; per half,
  the 7 Mi accumulate in 7 PSUM banks (8 DoubleRow matmuls each, FD=512,
  1 LDW per MM - measured free at FD=512), recombined while the next half
  streams. Final rowscale on ACT (scale=rs), alpha on DVE, DMA out.
"""

import numpy as np

import concourse.bacc as bacc
import concourse.bass as bass
import concourse.mybir as mybir
import concourse.tile as tile
from concourse.bass import ts

F32 = mybir.dt.float32
FP8 = mybir.dt.float8e4
AOP = mybir.AluOpType
ACTF = mybir.ActivationFunctionType
DR = mybir.MatmulPerfMode.DoubleRow

P = 128
OT = 512             # psum tile width (one fp32 bank)
N_WARM = 24          # junk matmuls to ramp the PE clock gate


def build_nc(M, IN_F, O_SH, with_bias):
    """Per-core SPMD program; shapes are per-core shard shapes."""
    MH, KH, NH = M // 2, IN_F // 2, O_SH // 2
    NRB = MH // P            # row-blocks (top+bottom pair each)
    KS = KH // P             # k-subtiles per Strassen operand
    NKK = KS // 2            # DoubleRow passes per Mi
    IK = 7 * KS              # stationary free rows per row-block
    NBLK = M // P            # for rs / out indexing
    assert KS % 2 == 0 and NH == 2 * OT

    nc = bacc.Bacc("TRN2", target_bir_lowering=False, debug=False)

    # a8t: pre-tiled Strassen A-combos; row r*P+p holds, for i in 0..7, ko in
    # 0..KS, the 128 s-rows of row-block r from k-row ko*P+p of combo i.
    a8t = nc.dram_tensor("a8t", [NRB * P, IK * P], FP8, kind="ExternalInput").ap()
    # b8t: pre-tiled Strassen B-combos; row i*P+p holds, for h, kk, pr, the OT
    # o-columns of half h of combo i from k-row (2*kk+pr)*P+p.
    b8t = nc.dram_tensor(
        "b8t", [7 * P, 2 * NKK * 2 * OT], FP8, kind="ExternalInput"
    ).ap()
    rs = nc.dram_tensor("rs", [P, NBLK], F32, kind="ExternalInput").ap()
    alpha = nc.dram_tensor("alpha", [1, O_SH], F32, kind="ExternalInput").ap()
    if with_bias:
        bias = nc.dram_tensor("bias", [1, O_SH], F32, kind="ExternalInput").ap()
    out = nc.dram_tensor("out", [M, O_SH], F32, kind="ExternalOutput").ap()

    a_r = a8t.rearrange("(r p) (ik s) -> r p ik s", p=P, ik=IK)
    b_r = b8t.rearrange("(i p) (h kk pr c) -> i p h kk pr c", p=P, h=2, kk=NKK, pr=2)
    out_r = out.rearrange("(t p) o -> p t o", p=P)    # [128, NBLK, O_SH]

    with tile.TileContext(nc) as tc:
        with (
            tc.tile_pool(name="const", bufs=1) as constp,
            tc.tile_pool(name="bpool", bufs=1) as bpool,
            tc.tile_pool(name="apool", bufs=2) as apool,
            tc.tile_pool(name="cpool", bufs=2) as cpool,
            tc.tile_pool(name="opool", bufs=4) as opool,
            tc.tile_pool(name="psum", bufs=8, space="PSUM") as psum,
        ):
            b_tiles = [
                [
                    bpool.tile([P, NKK, 2, OT], FP8, tag=f"b{i}_{h}", name=f"b{i}_{h}")
                    for h in range(2)
                ]
                for i in range(7)
            ]

            # PE warm-up: junk matmuls (values never read); only dep is the
            # t=0 memset, so they dispatch at engine init and the HAM clock
            # gate is at 8/8 by the first real matmul (which also waits on
            # the first B tiles, ~14us in).
            junk = constp.tile([P, 2, OT], FP8, tag="junk")
            nc.vector.memset(junk[:], 0.0)
            ps_warm = psum.tile([P, OT], F32, tag="ps", name="ps_warm")
            for _ in range(N_WARM):
                nc.tensor.matmul(
                    ps_warm[:], junk[:, :, 0:P], junk[:],
                    start=True, stop=True, perf_mode=DR,
                )

            # Startup DMA. Sync queue: A row-blocks + outputs. Scalar queue:
            # B combos in consumption order (all h0 tiles i=0..6, then h1),
            # then rs/alpha (needed only at the first eviction, ~35us in).
            def load_a(r, nsplit=4):
                at = apool.tile([P, IK, P], FP8, tag="a", name=f"a_{r}")
                js = IK // nsplit
                for j in range(nsplit):
                    nc.sync.dma_start(
                        at[:, j * js : (j + 1) * js, :],
                        a_r[r, :, j * js : (j + 1) * js, :],
                    )
                return at

            at_tiles = {}

            def ensure_a(r, nsplit=4):
                if r not in at_tiles:
                    at_tiles[r] = load_a(r, nsplit)

            ensure_a(0, nsplit=8)
            rs_sb = constp.tile([P, NBLK], F32, tag="rs_sb")
            nc.scalar.dma_start(rs_sb[:], rs[:, :])
            for h in range(2):
                for i in range(7):
                    nc.scalar.dma_start(b_tiles[i][h][:], b_r[i, :, h])
            ensure_a(1)
            alpha_bc = constp.tile([P, O_SH], F32, tag="alpha_bc")
            nc.scalar.dma_start(alpha_bc[:], alpha[0:1, :].to_broadcast((P, O_SH)))
            if with_bias:
                bias_bc = constp.tile([P, O_SH], F32, tag="bias_bc")
                nc.scalar.dma_start(bias_bc[:], bias[0:1, :].to_broadcast((P, O_SH)))

            # First two row-blocks run h0 before either h1 so the B h1-half
            # DMAs get extra time to land.
            sched = [(0, 0), (1, 0), (0, 1), (1, 1)] + [
                (r, h) for r in range(2, NRB) for h in range(2)
            ]
            c_tiles = {}
            TT = nc.vector.tensor_tensor
            ACT = nc.scalar.activation

            for j, (r, h) in enumerate(sched):
                for rr, _ in sched[j + 1 : j + 3]:
                    ensure_a(rr)
                at = at_tiles[r]
                if r not in c_tiles:
                    c_tiles[r] = (
                        cpool.tile([P, O_SH], F32, tag="top", name=f"top_{r}"),
                        cpool.tile([P, O_SH], F32, tag="bot", name=f"bot_{r}"),
                    )
                top, bot = c_tiles[r]

                ps = [
                    psum.tile([P, OT], F32, tag="ps", name=f"ps_{r}_{h}_{i}")
                    for i in range(7)
                ]
                for i in range(7):
                    for kk in range(NKK):
                        nc.tensor.matmul(
                            ps[i][:],
                            at[:, i * KS + 2 * kk : i * KS + 2 * kk + 2, :],
                            b_tiles[i][h][:, kk, :, :],
                            start=(kk == 0),
                            stop=(kk == NKK - 1),
                            perf_mode=DR,
                        )

                # C11 = M1+M4-M5+M7  C12 = M3+M5  C21 = M2+M4  C22 = M1-M2+M3+M6
                # (ps[i] = M_{i+1}).  top row-block = [C11 | C12], bottom =
                # [C21 | C22].  PSUM banks are single-ported: ScalarE and
                # VectorE must never touch the same bank concurrently, so the
                # banks are partitioned by reader engine - ACT reads ONLY
                # M1/M2, DVE reads ONLY M3..M7.  DVE order frees banks in
                # the order the next half re-needs them (~1.7us apart).
                hl = ts(h, OT)                              # C11/C21 cols
                hr = slice(NH + h * OT, NH + (h + 1) * OT)  # C12/C22 cols
                ACT(top[:, hl], ps[0][:], ACTF.Copy)            # top_l = M1
                ACT(bot[:, hl], ps[1][:], ACTF.Copy)            # bot_l = M2
                TT(bot[:, hr], top[:, hl], bot[:, hl], AOP.subtract)  # M1-M2
                nc.vector.tensor_copy(top[:, hr], ps[2][:])     # top_r = M3
                TT(bot[:, hr], bot[:, hr], ps[2][:], AOP.add)   # += M3
                TT(top[:, hl], top[:, hl], ps[3][:], AOP.add)   # += M4
                TT(bot[:, hl], bot[:, hl], ps[3][:], AOP.add)   # += M4
                TT(top[:, hl], top[:, hl], ps[4][:], AOP.subtract)  # -= M5
                TT(top[:, hr], top[:, hr], ps[4][:], AOP.add)   # += M5
                TT(bot[:, hr], bot[:, hr], ps[5][:], AOP.add)   # += M6
                TT(top[:, hl], top[:, hl], ps[6][:], AOP.add)   # += M7

                # Each half's four 512-col C slices are final once its
                # C-adds are done - evict immediately (shrinks the end-of-
                # kernel tail and spreads ACT/DVE/DMA eviction work).
                for tile_, tidx in ((top, r), (bot, NRB + r)):
                    for sl in (hl, hr):
                        ot_t = opool.tile([P, OT], F32, tag="ot")
                        ACT(
                            ot_t[:], tile_[:, sl], ACTF.Copy,
                            scale=rs_sb[:, tidx : tidx + 1],
                        )
                        TT(ot_t[:], ot_t[:], alpha_bc[:, sl], AOP.mult)
                        if with_bias:
                            TT(ot_t[:], ot_t[:], bias_bc[:, sl], AOP.add)
                        nc.sync.dma_start(out_r[:, tidx, sl], ot_t[:])
                if h == 1:
                    del c_tiles[r]

    nc.compile()
    return nc


def host_prep(x, weight, bias, n_cores):
    """Host-side quantize + Strassen combos + pre-tiled layout prep."""
    import ml_dtypes

    IN_F = x.shape[-1]
    OUT_F = weight.shape[0]
    M = int(np.prod(x.shape[:-1]))
    O_SH = OUT_F // n_cores
    MH, KH, NH = M // 2, IN_F // 2, O_SH // 2
    NRB = MH // P
    KS = KH // P
    NKK = KS // 2
    NBLK = M // P

    # e4m3 codes for ints -14..14 (all exactly representable); index v+14.
    lut = (
        np.arange(-14, 15, dtype=np.float32)
        .astype(ml_dtypes.float8_e4m3)
        .view(np.uint8)
    )

    x2 = x.reshape(M, IN_F)
    maxabs = np.maximum(np.abs(x2).max(axis=1), 1e-6).astype(np.float32)
    rs = (maxabs / np.float32(7.0)).astype(np.float32)
    rs_striped = np.ascontiguousarray(rs.reshape(NBLK, P).T)  # [128, NBLK]

    qi = np.rint(x2 * (np.float32(7.0) / maxabs)[:, None]).astype(np.int8)
    A11, A12 = qi[:MH, :KH], qi[:MH, KH:]
    A21, A22 = qi[MH:, :KH], qi[MH:, KH:]
    AM = np.stack(
        [A11 + A22, A21 + A22, A11, A22, A11 + A12, A21 - A11, A12 - A22]
    )  # [7, MH, KH], |.| <= 14
    acodes = lut[(AM + 14).astype(np.uint8)]
    a8t = (
        acodes.reshape(7, NRB, P, KS, P)   # (i, r, s, ko, p)
        .transpose(1, 4, 0, 3, 2)          # (r, p, i, ko, s)
        .reshape(NRB * P, 7 * KS * P)
    )
    a8t = np.ascontiguousarray(a8t).view(ml_dtypes.float8_e4m3)

    thresh = np.float32(0.05) * np.float32(np.abs(weight).mean(dtype=np.float64))
    with_bias = bool(np.any(bias))

    in_maps = []
    for c in range(n_cores):
        o0, o1 = c * O_SH, (c + 1) * O_SH
        w_sh = weight[o0:o1]
        si = np.where(
            np.abs(w_sh) < thresh, np.int8(0), np.sign(w_sh).astype(np.int8)
        )  # [O_SH, IN_F]
        # B[k, o] = si[o, k]; 2x2 blocks of [KH, NH]
        B11, B12 = si[:NH, :KH].T, si[NH:, :KH].T
        B21, B22 = si[:NH, KH:].T, si[NH:, KH:].T
        BM = np.stack(
            [B11 + B22, B11, B12 - B22, B21 - B11, B22, B11 + B12, B21 + B22]
        )  # [7, KH, NH], |.| <= 2
        bcodes = lut[(BM + 14).astype(np.uint8)]
        b8t = (
            bcodes.reshape(7, NKK, 2, P, 2, OT)   # (i, kk, pr, p, h, c)
            .transpose(0, 3, 4, 1, 2, 5)          # (i, p, h, kk, pr, c)
            .reshape(7 * P, 2 * NKK * 2 * OT)
        )
        b8t = np.ascontiguousarray(b8t).view(ml_dtypes.float8_e4m3)
        m = {
            "a8t": a8t,
            "b8t": b8t,
            "rs": rs_striped,
            "alpha": np.abs(w_sh).mean(axis=1, dtype=np.float32).reshape(1, O_SH),
        }
        if with_bias:
            m["bias"] = np.ascontiguousarray(bias[o0:o1], dtype=np.float32).reshape(
                1, O_SH
            )
        in_maps.append(m)
    return in_maps, with_bias


_NC_CACHE = {}


def _get_nc(M, IN_F, O_SH, with_bias):
    key = (M, IN_F, O_SH, with_bias)
    if key not in _NC_CACHE:
        _NC_CACHE[key] = build_nc(M, IN_F, O_SH, with_bias)
    return _NC_CACHE[key]


def kernel(x, weight, bias, _trace=False):
    from concourse.bass_utils import run_bass_kernel_spmd

    N_CORES = 8
    x = np.asarray(x)
    weight = np.asarray(weight)
    bias = np.asarray(bias)
    IN_F = x.shape[-1]
    OUT_F = weight.shape[0]
    M = int(np.prod(x.shape[:-1]))
    O_SH = OUT_F // N_CORES

    in_maps, with_bias = host_prep(x, weight, bias, N_CORES)
    nc = _get_nc(M, IN_F, O_SH, with_bias)
    res = run_bass_kernel_spmd(
        nc, in_maps, core_ids=list(range(N_CORES)), trace=_trace
    )
    parts = [res.results[c]["out"].reshape(*x.shape[:-1], O_SH) for c in range(N_CORES)]
    full = np.concatenate(parts, axis=-1)
    if with_bias is False and np.any(bias):  # pragma: no cover (safety)
        full = full + bias
    if _trace:
        return full, res
    return full


# revision 16
# speedup vs baseline: 1.0345x; 1.0345x over previous
"""BitLinear (int4-fakequant x @ ternary-weight linear) Trainium2 Bass kernel.

Strassen variant. Math (per reference):
  maxabs[s] = max(|x[s, :]|) clamped to >= 1e-6
  q[s, k]   = round(x[s, k] / maxabs[s] * 7)           # in [-7, 7]
  xq        = q * maxabs / 7
  thresh    = 0.05 * mean(|w|)                          # global scalar
  sign[o,k] = 0 if |w[o,k]| < thresh else sign(w[o,k])  # in {-1, 0, 1}
  alpha[o]  = mean(|w[o, :]|)
  out[s, o] = (maxabs[s]/7) * alpha[o] * S[s,o] + bias[o],  S = q @ sign.T

S = A @ B with A = q [M, K] (ints in [-7,7]) and B = sign.T [K, O_SH] (ternary)
is computed with ONE level of Strassen: A, B split 2x2 into [M/2, K/2] and
[K/2, N/2] blocks, 7 products Mi instead of 8 -> 7/8 the PE-array work, which
is the binding resource (fp8 DoubleRow streams 1 col-pair/cycle = 157 TF/s;
the dense kernel measured 905 us vs the 874 us stream floor). Host precomputes
the O(n^2) part: int4/ternary quantization AND the Strassen input combos
(A11+A22 etc., |.|<=14; B combos |.|<=2 - all exactly representable in e4m3),
shipped pre-tiled to SBUF layout. |Mi| <= 28*2048 << 2^24 so fp32 PSUM
accumulation is EXACT; the C recombination is spread over Scalar (3 PSUM->SBUF
copies), GpSimd (1 SBUF add) and Vector (7 PSUM adds, ordered so each PSUM
bank frees just before the next half re-needs it), all overlapped with the PE
stream. Column-parallel over out_f across 8 cores.

Device per-core schedule (M=8192, K=4096, O_SH=2048):
  B combos (7 x 2 o-halves x [128, 8, 2, 512] e4m3 = 112 KB/partition) are
  SBUF-resident, loaded in first-use order. Loop over 32 row-blocks (128 top
  rows r*128.. paired with 128 bottom rows 4096+r*128..; A combos for both
  land as one 1.75 MB pre-tiled DMA). Per row-block, 2 o-halves; per half,
  the 7 Mi accumulate in 7 PSUM banks (8 DoubleRow matmuls each, FD=512,
  1 LDW per MM - measured free at FD=512), recombined while the next half
  streams. Final rowscale on ACT (scale=rs), alpha on DVE, DMA out.
"""

import numpy as np

import concourse.bacc as bacc
import concourse.bass as bass
import concourse.mybir as mybir
import concourse.tile as tile
from concourse.bass import ts

F32 = mybir.dt.float32
FP8 = mybir.dt.float8e4
AOP = mybir.AluOpType
ACTF = mybir.ActivationFunctionType
DR = mybir.MatmulPerfMode.DoubleRow

P = 128
OT = 512             # psum tile width (one fp32 bank)
N_WARM = 24          # junk matmuls to ramp the PE clock gate


def build_nc(M, IN_F, O_SH, with_bias):
    """Per-core SPMD program; shapes are per-core shard shapes."""
    MH, KH, NH = M // 2, IN_F // 2, O_SH // 2
    NRB = MH // P            # row-blocks (top+bottom pair each)
    KS = KH // P             # k-subtiles per Strassen operand
    NKK = KS // 2            # DoubleRow passes per Mi
    IK = 7 * KS              # stationary free rows per row-block
    NBLK = M // P            # for rs / out indexing
    assert KS % 2 == 0 and NH == 2 * OT

    nc = bacc.Bacc("TRN2", target_bir_lowering=False, debug=False)

    # a8t: pre-tiled Strassen A-combos; row r*P+p holds, for i in 0..7, ko in
    # 0..KS, the 128 s-rows of row-block r from k-row ko*P+p of combo i.
    a8t = nc.dram_tensor("a8t", [NRB * P, IK * P], FP8, kind="ExternalInput").ap()
    # b8t: pre-tiled Strassen B-combos; row i*P+p holds, for h, kk, pr, the OT
    # o-columns of half h of combo i from k-row (2*kk+pr)*P+p.
    b8t = nc.dram_tensor(
        "b8t", [7 * P, 2 * NKK * 2 * OT], FP8, kind="ExternalInput"
    ).ap()
    rs = nc.dram_tensor("rs", [P, NBLK], F32, kind="ExternalInput").ap()
    alpha = nc.dram_tensor("alpha", [1, O_SH], F32, kind="ExternalInput").ap()
    if with_bias:
        bias = nc.dram_tensor("bias", [1, O_SH], F32, kind="ExternalInput").ap()
    out = nc.dram_tensor("out", [M, O_SH], F32, kind="ExternalOutput").ap()

    a_r = a8t.rearrange("(r p) (ik s) -> r p ik s", p=P, ik=IK)
    b_r = b8t.rearrange("(i p) (h kk pr c) -> i p h kk pr c", p=P, h=2, kk=NKK, pr=2)
    out_r = out.rearrange("(t p) o -> p t o", p=P)    # [128, NBLK, O_SH]

    with tile.TileContext(nc) as tc:
        with (
            tc.tile_pool(name="const", bufs=1) as constp,
            tc.tile_pool(name="bpool", bufs=1) as bpool,
            tc.tile_pool(name="apool", bufs=2) as apool,
            tc.tile_pool(name="cpool", bufs=2) as cpool,
            tc.tile_pool(name="opool", bufs=2) as opool,
            tc.tile_pool(name="psum", bufs=8, space="PSUM") as psum,
        ):
            b_tiles = [
                [
                    bpool.tile([P, NKK, 2, OT], FP8, tag=f"b{i}_{h}", name=f"b{i}_{h}")
                    for h in range(2)
                ]
                for i in range(7)
            ]

            # PE warm-up: junk matmuls (values never read); only dep is the
            # t=0 memset, so they dispatch at engine init and the HAM clock
            # gate is at 8/8 by the first real matmul (which also waits on
            # the first B tiles, ~14us in).
            junk = constp.tile([P, 2, OT], FP8, tag="junk")
            nc.vector.memset(junk[:], 0.0)
            ps_warm = psum.tile([P, OT], F32, tag="ps", name="ps_warm")
            for _ in range(N_WARM):
                nc.tensor.matmul(
                    ps_warm[:], junk[:, :, 0:P], junk[:],
                    start=True, stop=True, perf_mode=DR,
                )

            # Startup DMA. Sync queue: A row-blocks + outputs. Scalar queue:
            # B combos in consumption order (all h0 tiles i=0..6, then h1),
            # then rs/alpha (needed only at the first eviction, ~35us in).
            def load_a(r, nsplit=4):
                at = apool.tile([P, IK, P], FP8, tag="a", name=f"a_{r}")
                js = IK // nsplit
                for j in range(nsplit):
                    nc.sync.dma_start(
                        at[:, j * js : (j + 1) * js, :],
                        a_r[r, :, j * js : (j + 1) * js, :],
                    )
                return at

            at_tiles = {}

            def ensure_a(r, nsplit=4):
                if r not in at_tiles:
                    at_tiles[r] = load_a(r, nsplit)

            ensure_a(0, nsplit=8)
            rs_sb = constp.tile([P, NBLK], F32, tag="rs_sb")
            nc.scalar.dma_start(rs_sb[:], rs[:, :])
            for h in range(2):
                for i in range(7):
                    nc.scalar.dma_start(b_tiles[i][h][:], b_r[i, :, h])
            ensure_a(1)
            alpha_bc = constp.tile([P, O_SH], F32, tag="alpha_bc")
            nc.scalar.dma_start(alpha_bc[:], alpha[0:1, :].to_broadcast((P, O_SH)))
            if with_bias:
                bias_bc = constp.tile([P, O_SH], F32, tag="bias_bc")
                nc.scalar.dma_start(bias_bc[:], bias[0:1, :].to_broadcast((P, O_SH)))

            # First two row-blocks run h0 before either h1 so the B h1-half
            # DMAs get extra time to land.
            sched = [(0, 0), (1, 0), (0, 1), (1, 1)] + [
                (r, h) for r in range(2, NRB) for h in range(2)
            ]
            c_tiles = {}
            TT = nc.vector.tensor_tensor
            ACT = nc.scalar.activation

            for j, (r, h) in enumerate(sched):
                for rr, _ in sched[j + 1 : j + 3]:
                    ensure_a(rr)
                at = at_tiles[r]
                if r not in c_tiles:
                    c_tiles[r] = (
                        cpool.tile([P, O_SH], F32, tag="top", name=f"top_{r}"),
                        cpool.tile([P, O_SH], F32, tag="bot", name=f"bot_{r}"),
                    )
                top, bot = c_tiles[r]

                ps = [
                    psum.tile([P, OT], F32, tag="ps", name=f"ps_{r}_{h}_{i}")
                    for i in range(7)
                ]
                for i in range(7):
                    for kk in range(NKK):
                        nc.tensor.matmul(
                            ps[i][:],
                            at[:, i * KS + 2 * kk : i * KS + 2 * kk + 2, :],
                            b_tiles[i][h][:, kk, :, :],
                            start=(kk == 0),
                            stop=(kk == NKK - 1),
                            perf_mode=DR,
                        )

                # C11 = M1+M4-M5+M7  C12 = M3+M5  C21 = M2+M4  C22 = M1-M2+M3+M6
                # (ps[i] = M_{i+1}).  top row-block = [C11 | C12], bottom =
                # [C21 | C22].  PSUM banks are single-ported: ScalarE and
                # VectorE must never touch the same bank concurrently, so the
                # banks are partitioned by reader engine - ACT reads ONLY
                # M1/M2, DVE reads ONLY M3..M7.  DVE order frees banks in
                # the order the next half re-needs them (~1.7us apart).
                hl = ts(h, OT)                              # C11/C21 cols
                hr = slice(NH + h * OT, NH + (h + 1) * OT)  # C12/C22 cols
                ACT(top[:, hl], ps[0][:], ACTF.Copy)            # top_l = M1
                ACT(bot[:, hl], ps[1][:], ACTF.Copy)            # bot_l = M2
                TT(bot[:, hr], top[:, hl], bot[:, hl], AOP.subtract)  # M1-M2
                nc.vector.tensor_copy(top[:, hr], ps[2][:])     # top_r = M3
                TT(bot[:, hr], bot[:, hr], ps[2][:], AOP.add)   # += M3
                TT(top[:, hl], top[:, hl], ps[3][:], AOP.add)   # += M4
                TT(bot[:, hl], bot[:, hl], ps[3][:], AOP.add)   # += M4
                TT(top[:, hl], top[:, hl], ps[4][:], AOP.subtract)  # -= M5
                TT(top[:, hr], top[:, hr], ps[4][:], AOP.add)   # += M5
                TT(bot[:, hr], bot[:, hr], ps[5][:], AOP.add)   # += M6
                TT(top[:, hl], top[:, hl], ps[6][:], AOP.add)   # += M7

                if h == 1:
                    for tile_, tidx in ((top, r), (bot, NRB + r)):
                        ot_t = opool.tile([P, O_SH], F32, tag="ot", name=f"ot_{tidx}")
                        ACT(
                            ot_t[:], tile_[:], ACTF.Copy,
                            scale=rs_sb[:, tidx : tidx + 1],
                        )
                        TT(ot_t[:], ot_t[:], alpha_bc[:], AOP.mult)
                        if with_bias:
                            TT(ot_t[:], ot_t[:], bias_bc[:], AOP.add)
                        nc.sync.dma_start(out_r[:, tidx, :], ot_t[:])
                    del c_tiles[r]

    nc.compile()
    return nc


def host_prep(x, weight, bias, n_cores):
    """Host-side quantize + Strassen combos + pre-tiled layout prep."""
    import ml_dtypes

    IN_F = x.shape[-1]
    OUT_F = weight.shape[0]
    M = int(np.prod(x.shape[:-1]))
    O_SH = OUT_F // n_cores
    MH, KH, NH = M // 2, IN_F // 2, O_SH // 2
    NRB = MH // P
    KS = KH // P
    NKK = KS // 2
    NBLK = M // P

    # e4m3 codes for ints -14..14 (all exactly representable); index v+14.
    lut = (
        np.arange(-14, 15, dtype=np.float32)
        .astype(ml_dtypes.float8_e4m3)
        .view(np.uint8)
    )

    x2 = x.reshape(M, IN_F)
    maxabs = np.maximum(np.abs(x2).max(axis=1), 1e-6).astype(np.float32)
    rs = (maxabs / np.float32(7.0)).astype(np.float32)
    rs_striped = np.ascontiguousarray(rs.reshape(NBLK, P).T)  # [128, NBLK]

    qi = np.rint(x2 * (np.float32(7.0) / maxabs)[:, None]).astype(np.int8)
    A11, A12 = qi[:MH, :KH], qi[:MH, KH:]
    A21, A22 = qi[MH:, :KH], qi[MH:, KH:]
    AM = np.stack(
        [A11 + A22, A21 + A22, A11, A22, A11 + A12, A21 - A11, A12 - A22]
    )  # [7, MH, KH], |.| <= 14
    acodes = lut[(AM + 14).astype(np.uint8)]
    a8t = (
        acodes.reshape(7, NRB, P, KS, P)   # (i, r, s, ko, p)
        .transpose(1, 4, 0, 3, 2)          # (r, p, i, ko, s)
        .reshape(NRB * P, 7 * KS * P)
    )
    a8t = np.ascontiguousarray(a8t).view(ml_dtypes.float8_e4m3)

    thresh = np.float32(0.05) * np.float32(np.abs(weight).mean(dtype=np.float64))
    with_bias = bool(np.any(bias))

    in_maps = []
    for c in range(n_cores):
        o0, o1 = c * O_SH, (c + 1) * O_SH
        w_sh = weight[o0:o1]
        si = np.where(
            np.abs(w_sh) < thresh, np.int8(0), np.sign(w_sh).astype(np.int8)
        )  # [O_SH, IN_F]
        # B[k, o] = si[o, k]; 2x2 blocks of [KH, NH]
        B11, B12 = si[:NH, :KH].T, si[NH:, :KH].T
        B21, B22 = si[:NH, KH:].T, si[NH:, KH:].T
        BM = np.stack(
            [B11 + B22, B11, B12 - B22, B21 - B11, B22, B11 + B12, B21 + B22]
        )  # [7, KH, NH], |.| <= 2
        bcodes = lut[(BM + 14).astype(np.uint8)]
        b8t = (
            bcodes.reshape(7, NKK, 2, P, 2, OT)   # (i, kk, pr, p, h, c)
            .transpose(0, 3, 4, 1, 2, 5)          # (i, p, h, kk, pr, c)
            .reshape(7 * P, 2 * NKK * 2 * OT)
        )
        b8t = np.ascontiguousarray(b8t).view(ml_dtypes.float8_e4m3)
        m = {
            "a8t": a8t,
            "b8t": b8t,
            "rs": rs_striped,
            "alpha": np.abs(w_sh).mean(axis=1, dtype=np.float32).reshape(1, O_SH),
        }
        if with_bias:
            m["bias"] = np.ascontiguousarray(bias[o0:o1], dtype=np.float32).reshape(
                1, O_SH
            )
        in_maps.append(m)
    return in_maps, with_bias


_NC_CACHE = {}


def _get_nc(M, IN_F, O_SH, with_bias):
    key = (M, IN_F, O_SH, with_bias)
    if key not in _NC_CACHE:
        _NC_CACHE[key] = build_nc(M, IN_F, O_SH, with_bias)
    return _NC_CACHE[key]


def kernel(x, weight, bias, _trace=False):
    from concourse.bass_utils import run_bass_kernel_spmd

    N_CORES = 8
    x = np.asarray(x)
    weight = np.asarray(weight)
    bias = np.asarray(bias)
    IN_F = x.shape[-1]
    OUT_F = weight.shape[0]
    M = int(np.prod(x.shape[:-1]))
    O_SH = OUT_F // N_CORES

    in_maps, with_bias = host_prep(x, weight, bias, N_CORES)
    nc = _get_nc(M, IN_F, O_SH, with_bias)
    res = run_bass_kernel_spmd(
        nc, in_maps, core_ids=list(range(N_CORES)), trace=_trace
    )
    parts = [res.results[c]["out"].reshape(*x.shape[:-1], O_SH) for c in range(N_CORES)]
    full = np.concatenate(parts, axis=-1)
    if with_bias is False and np.any(bias):  # pragma: no cover (safety)
        full = full + bias
    if _trace:
        return full, res
    return full
